# revision 12
# baseline (speedup 1.0000x reference)
"""DKMIL Trainium2 kernel: 8-core SPMD Bass/Tile implementation.

Sharding: data-parallel over the 2048 bag instances (256 rows/core); knowledge
bag is processed by the same conv path (256 KB rows/core) and AllGathered.
Column softmaxes in skip blocks run row-sharded with AllReduce(max/add).
Top-S selection via rank computation + one-hot matmul gather (no sort needed).

Walrus in this environment allows only ONE sync-wait per instruction; the
`Ob` helper issues tiny same-engine "observer" ops so every real instruction
needs at most one unobserved semaphore tick.
"""
import numpy as np

NC = 8
N = 2048
NL = 256            # local rows per core
S = 204
BN_SCALE = 1.0 / float(np.sqrt(1.0 + 1e-5))
F32NP = np.float32

# ---------------------------------------------------------------- host prep --
def _conv1_wg(w1, b1):
    out = np.zeros((169, 960), F32NP)
    for o in range(20):
        for yp in range(2):
            for x in range(24):
                col = o * 48 + yp * 24 + x
                for dy in range(5):
                    for dx in range(5):
                        out[(yp + dy) * 28 + (x + dx), col] = w1[o, 0, dy, dx]
                out[168, col] = b1[o]
    return out


def _conv2_wy(w2, b2):
    out = np.zeros((1201, 400), F32NP)
    for o in range(50):
        for x in range(8):
            col = o * 8 + x
            for c in range(20):
                for dy in range(5):
                    for dx in range(5):
                        out[dy * 240 + c * 12 + (x + dx), col] = w2[o, c, dy, dx]
            out[1200, col] = b2[o]
    return out


def _emb_wr(ew, eb):
    out = np.zeros((801, 256), F32NP)
    for yp in range(4):
        for o in range(50):
            for xp in range(4):
                out[yp * 200 + o * 4 + xp, :] = ew[:, o * 16 + yp * 4 + xp]
    out[800, :] = eb
    return out


def _wb(wT, b):
    return np.concatenate([wT, b[None, :]], axis=0).astype(F32NP)


def host_prep(inputs):
    g = {k: np.asarray(v, F32NP) for k, v in inputs.items()}
    h = {}
    h['w1g'] = _conv1_wg(g['dt_c1_w'], g['dt_c1_b'])
    h['w2y'] = _conv2_wy(g['dt_c2_w'], g['dt_c2_b'])
    h['ewr'] = _emb_wr(g['dt_emb_w'], g['dt_emb_b'])
    for sk in ('sk1', 'sk2'):
        h[sk + '_rhs'] = _wb(
            np.concatenate([g[sk + '_w1'].T, g[sk + '_w2'].T, g[sk + '_w3'].T], 1),
            np.concatenate([g[sk + '_b1'], g[sk + '_b2'], g[sk + '_b3']]))
    h['sk3_rhs12'] = _wb(np.concatenate([g['sk3_w1'].T, g['sk3_w2'].T], 1),
                         np.concatenate([g['sk3_b1'], g['sk3_b2']]))
    h['sk3_w3T'] = _wb(g['sk3_w3'].T, g['sk3_b3'])
    h['mk_rhs12'] = _wb(np.concatenate([g['mk_w1'].T, g['mk_w2'].T], 1),
                        np.concatenate([g['mk_b1'], g['mk_b2']]))
    h['mk_w3T'] = _wb(g['mk_w3'].T, g['mk_b3'])
    h['w4col'] = g['mk_w4'][0][:, None].copy()
    h['fuse_w1T'] = _wb(g['fuse_w'][:, :256].T, g['fuse_b'])
    h['fw2T'] = np.ascontiguousarray(g['fuse_w'][:, 256:].T)
    h['a1feT'] = _wb(g['a1_fe_w'].T, g['a1_fe_b'])
    h['a1vuT'] = _wb(np.concatenate([g['a1_v_w'].T, g['a1_u_w'].T], 1),
                     np.concatenate([g['a1_v_b'], g['a1_u_b']]))
    h['aw1col'] = g['a1_aw_w'][0][:, None].copy()
    h['a1embT'] = _wb(g['a1_emb_w'].T, g['a1_emb_b'])
    h['a2feT'] = _wb(g['a2_fe_w'].T, g['a2_fe_b'])
    h['a2vuT'] = _wb(np.concatenate([g['a2_v_w'].T, g['a2_u_w'].T], 1),
                     np.concatenate([g['a2_v_b'], g['a2_u_b']]))
    h['aw2col'] = g['a2_aw_w'][0][:, None].copy()
    h['a2embT'] = _wb(g['a2_emb_w'].T, g['a2_emb_b'])
    h['sembT'] = _wb(g['semb_w'].T, g['semb_b'])
    h['clsT'] = _wb(g['cls_w'].T, g['cls_b'])
    h['ident'] = np.eye(128, dtype=F32NP)
    h['iota'] = np.arange(2048, dtype=F32NP)[None, :].copy()
    h['kidx'] = np.arange(204, dtype=F32NP)[:, None].copy()
    scal = {'b4': float(g['mk_b4'][0]), 'awb1': float(g['a1_aw_b'][0]),
            'awb2': float(g['a2_aw_b'][0])}
    B = g['B'].reshape(N, 784)
    KB = g['KB'].reshape(N, 784)
    percore = []
    for c in range(NC):
        d = dict(h)
        d['imgs'] = np.ascontiguousarray(
            np.concatenate([B[c * NL:(c + 1) * NL], KB[c * NL:(c + 1) * NL]], 0))
        d['jidx'] = (c * NL + np.arange(NL, dtype=F32NP))[:, None].copy()
        percore.append(d)
    return h, percore, scal


# ---------------------------------------------------------------- device prog --
def build_nc(scal):
    import concourse.bass as bass
    import concourse.bacc as bacc
    import concourse.tile as tile
    from concourse import mybir

    F32 = mybir.dt.float32
    AF = mybir.ActivationFunctionType
    ALU = mybir.AluOpType
    AX = mybir.AxisListType
    RG = [list(range(NC))]

    nc = bacc.Bacc("TRN2", num_devices=NC)

    def par(name, shape, out=False):
        return nc.declare_dram_parameter(name, list(shape), F32, isOutput=out)

    I = {}
    for nm, shp in [
        ('imgs', (512, 784)), ('jidx', (256, 1)), ('w1g', (169, 960)),
        ('w2y', (1201, 400)), ('ewr', (801, 256)), ('sk1_rhs', (257, 512)),
        ('sk2_rhs', (257, 512)), ('sk3_rhs12', (2049, 256)),
        ('sk3_w3T', (2049, 2048)), ('mk_rhs12', (2049, 256)),
        ('mk_w3T', (129, 2048)), ('w4col', (128, 1)), ('fuse_w1T', (257, 256)),
        ('fw2T', (204, 256)), ('a1feT', (257, 256)), ('a1vuT', (257, 256)),
        ('aw1col', (128, 1)), ('a1embT', (257, 256)), ('a2feT', (513, 256)),
        ('a2vuT', (257, 256)), ('aw2col', (128, 1)), ('a2embT', (257, 512)),
        ('sembT', (513, 256)), ('clsT', (257, 1)), ('ident', (128, 128)),
        ('iota', (1, 2048)), ('kidx', (204, 1)),
    ]:
        I[nm] = par(nm, shp)
    out_Y = par('Y', (1, 1), out=True)
    out_A = par('Apart', (256, 1), out=True)

    with tile.TileContext(nc) as tc:
      from contextlib import ExitStack
      with tc.tile_pool(name="persist", bufs=1) as P, \
           tc.tile_pool(name="work", bufs=2) as W, \
           tc.tile_pool(name="wload", bufs=3) as WL, \
           tc.tile_pool(name="ps", bufs=3, space="PSUM") as PS, \
           tc.tile_pool(name="psB", bufs=2, space="PSUM") as PSB, \
           tc.tile_pool(name="obsps", bufs=2, space="PSUM") as OPS, \
           tc.tile_pool(name="dram", bufs=1, space="DRAM") as DR:

        def one(ap):
            idx = (slice(0, 1),) + (0,) * (len(ap.shape) - 2) + (slice(0, 1),)
            return ap[idx]

        def pe_obs(*aps):
            for ap in aps:
                sc = OPS.tile([1, 1], F32, tag="ps")
                nc.tensor.matmul(sc, one(ap), one(ap), start=True, stop=True)

        dve_scr = P.tile([1, 1], F32, tag="dvescr")
        act_scr = P.tile([1, 1], F32, tag="actscr")

        def dve_obs(*aps):
            pass

        def act_obs(*aps):
            pass

        ident = P.tile([128, 128], F32, tag="ident")
        nc.sync.dma_start(out=ident, in_=I['ident'][:, :])
        ones1 = P.tile([1, 128], F32, tag="ones1")
        nc.vector.memset(ones1, 1.0)
        pe_obs(ident, ones1)

        def tr(dst, src_ap, scale=None):
            """PE transpose src [p,f] -> dst sbuf [f,p] via DVE drain."""
            p = src_ap.shape[0]
            f = src_ap.shape[-1]
            pe_obs(src_ap)
            tp = PS.tile([128, 128], F32, tag="ps")
            nc.tensor.transpose(tp[0:f, 0:p], src_ap, ident[0:p, 0:p])
            dve_obs(tp)
            if scale is None:
                nc.vector.tensor_copy(dst, tp[0:f, 0:p])
            else:
                nc.vector.tensor_scalar_mul(dst, tp[0:f, 0:p], scale)

        # ============================ conv stack =============================
        _cv = ExitStack()
        CV = _cv.enter_context(tc.tile_pool(name="cv", bufs=1))
        w1gA = CV.tile([84, 960], F32, tag="w1gA", bufs=1)
        nc.sync.dma_start(out=w1gA, in_=I['w1g'][0:84, :])
        w1gB = CV.tile([84, 960], F32, tag="w1gB", bufs=1)
        nc.sync.dma_start(out=w1gB, in_=I['w1g'][84:168, :])
        w1gBb = CV.tile([1, 960], F32, tag="w1gBb", bufs=1)
        nc.sync.dma_start(out=w1gBb, in_=I['w1g'][168:169, :])
        w2y = CV.tile([120, 10, 400], F32, tag="w2y", bufs=1)
        nc.sync.dma_start(out=w2y, in_=I['w2y'][0:1200, :].rearrange(
            "(c r) f -> r c f", c=10))
        w2yb = CV.tile([1, 400], F32, tag="w2yb", bufs=1)
        nc.sync.dma_start(out=w2yb, in_=I['w2y'][1200:1201, :])
        ewr = CV.tile([100, 8, 256], F32, tag="ewr", bufs=1)
        nc.sync.dma_start(out=ewr, in_=I['ewr'][0:800, :].rearrange(
            "(c r) f -> r c f", c=8))
        ewrb = CV.tile([1, 256], F32, tag="ewrb", bufs=1)
        nc.sync.dma_start(out=ewrb, in_=I['ewr'][800:801, :])
        pe_obs(w1gA, w1gB, w1gBb, w2y, w2yb, ewr, ewrb)

        dt = []  # 4 tiles [128, 256]
        for nb in range(4):
            img = CV.tile([128, 784], F32, tag="img", bufs=2)
            nc.sync.dma_start(out=img, in_=I['imgs'][128 * nb:128 * (nb + 1), :])
            pooled1 = CV.tile([128, 2880], F32, tag="pooled1")
            for gi in range(12):
                lA = CV.tile([84, 128], F32, tag="lA", bufs=2)
                tr(lA, img[:, 56 * gi:56 * gi + 84])
                lB = CV.tile([84, 128], F32, tag="lB", bufs=2)
                tr(lB, img[:, 56 * gi + 84:56 * gi + 168])
                for fc in range(2):
                    o1 = PS.tile([128, 480], F32, tag="ps")
                    pe_obs(lA, lB)
                    nc.tensor.matmul(o1, lA, w1gA[:, 480 * fc:480 * (fc + 1)],
                                     start=True, stop=False)
                    nc.tensor.matmul(o1, lB, w1gB[:, 480 * fc:480 * (fc + 1)],
                                     start=False, stop=False)
                    nc.tensor.matmul(o1, ones1, w1gBb[:, 480 * fc:480 * (fc + 1)],
                                     start=False, stop=True)
                    o1s = CV.tile([128, 480], F32, tag="o1s")
                    dve_obs(o1)
                    nc.vector.tensor_copy(o1s, o1)
                    o1v = o1s.rearrange("p (o y x) -> p o y x", o=10, y=2)
                    t1 = CV.tile([128, 10, 24], F32, tag="t1")
                    nc.vector.tensor_tensor(out=t1, in0=o1v[:, :, 0, :],
                                            in1=o1v[:, :, 1, :], op=ALU.max)
                    t1v = t1.rearrange("p o (x q) -> p o x q", q=2)
                    t2 = CV.tile([128, 10, 12], F32, tag="t2")
                    nc.vector.tensor_tensor(out=t2, in0=t1v[:, :, :, 0],
                                            in1=t1v[:, :, :, 1], op=ALU.max)
                    act_obs(t2)
                    nc.scalar.activation(
                        pooled1[:, 240 * gi + 120 * fc:240 * gi + 120 * (fc + 1)]
                        .rearrange("p (o x) -> p o x", o=10),
                        t2, AF.Lrelu, alpha=0.01)
            # conv2
            t2t = CV.tile([120, 24, 128], F32, tag="t2t")
            for t in range(24):
                tr(t2t[:, t, :], pooled1[:, 120 * t:120 * (t + 1)])
            pooled2 = CV.tile([128, 800], F32, tag="pooled2")
            prev = None
            for y in range(8):
                o2 = PS.tile([128, 400], F32, tag="ps")
                pe_obs(t2t[:, 2 * y, :])
                for ch in range(10):
                    nc.tensor.matmul(o2, t2t[:, 2 * y + ch, :], w2y[:, ch, :],
                                     start=(ch == 0), stop=False)
                nc.tensor.matmul(o2, ones1, w2yb, start=False, stop=True)
                if y % 2 == 0:
                    prev = CV.tile([128, 400], F32, tag="o2prev")
                    dve_obs(o2)
                    nc.vector.tensor_copy(prev, o2)
                else:
                    m = CV.tile([128, 400], F32, tag="o2m")
                    dve_obs(o2, prev)
                    nc.vector.tensor_tensor(out=m, in0=prev, in1=o2, op=ALU.max)
                    mv = m.rearrange("p (o x q) -> p o x q", o=50, q=2)
                    m2 = CV.tile([128, 50, 4], F32, tag="m2")
                    nc.vector.tensor_tensor(out=m2, in0=mv[:, :, :, 0],
                                            in1=mv[:, :, :, 1], op=ALU.max)
                    act_obs(m2)
                    h = y // 2
                    nc.scalar.activation(
                        pooled2[:, 200 * h:200 * (h + 1)]
                        .rearrange("p (o x) -> p o x", o=50),
                        m2, AF.Lrelu, alpha=0.01)
            t3 = CV.tile([100, 8, 128], F32, tag="t3")
            for t in range(8):
                tr(t3[:, t, :], pooled2[:, 100 * t:100 * (t + 1)])
            pe3 = PS.tile([128, 256], F32, tag="ps")
            pe_obs(t3[:, 0, :])
            for ch in range(8):
                nc.tensor.matmul(pe3, t3[:, ch, :], ewr[:, ch, :],
                                 start=(ch == 0), stop=False)
            nc.tensor.matmul(pe3, ones1, ewrb, start=False, stop=True)
            d = P.tile([128, 256], F32, tag=f"dt{nb}")
            act_obs(pe3)
            nc.scalar.activation(d, pe3, AF.Lrelu, alpha=0.01)
            dt.append(d)

        # transposes of dt: dtT[src][ch] [128, 2, 128] per src tile
        dtT = []
        for nb in range(4):
            tt = P.tile([128, 2, 128], F32, tag=f"dtT{nb}")
            for ch in range(2):
                tr(tt[:, ch, :], dt[nb][:, 128 * ch:128 * (ch + 1)])
            dtT.append(tt)
        _cv.close()

        # ===================== sk1/sk2 local X123 + AllGather ================
        _s2 = ExitStack()
        S2 = _s2.enter_context(tc.tile_pool(name="s2", bufs=2))
        skr = {}
        for nm in ('sk1_rhs', 'sk2_rhs'):
            a = S2.tile([128, 512], F32, tag=nm + "A", bufs=1)
            nc.sync.dma_start(out=a, in_=I[nm][0:128, :])
            b = S2.tile([128, 512], F32, tag=nm + "B", bufs=1)
            nc.sync.dma_start(out=b, in_=I[nm][128:256, :])
            c = S2.tile([1, 512], F32, tag=nm + "C", bufs=1)
            nc.sync.dma_start(out=c, in_=I[nm][256:257, :])
            skr[nm] = (a, b, c)

        ag1_in = DR.tile([256, 1024], F32)
        x123 = {1: [], 2: []}  # per sk: [tile [128,512] x2]
        for sk in (1, 2):
            a, b, c = skr[f'sk{sk}_rhs']
            for nt in range(2):
                src = nt if sk == 1 else nt + 2
                psx = PS.tile([128, 512], F32, tag="ps")
                pe_obs(dtT[src], a, b, c)
                nc.tensor.matmul(psx, dtT[src][:, 0, :], a, start=True, stop=False)
                nc.tensor.matmul(psx, dtT[src][:, 1, :], b, start=False, stop=False)
                nc.tensor.matmul(psx, ones1, c, start=False, stop=True)
                xt = S2.tile([128, 512], F32, tag=f"x123_{sk}_{nt}", bufs=1)
                act_obs(psx)
                nc.scalar.activation(xt, psx, AF.Lrelu, scale=BN_SCALE, alpha=0.01)
                x123[sk].append(xt)
                nc.sync.dma_start(
                    out=ag1_in[128 * nt:128 * (nt + 1),
                               512 * (sk - 1):512 * sk], in_=xt)
        ag1_out = DR.tile([2048, 1024], F32)
        nc.gpsimd.collective_compute("AllGather", ALU.bypass, replica_groups=RG,
                                     ins=[ag1_in.opt()], outs=[ag1_out.opt()])

        # =========== skip softmax for sk1/sk2 (column softmax via AR) ========
        def skip_softmax_out(sk):
            """Returns Xs_loc tiles [128,256] x2 for skip block sk (1|2)."""
            # X1 local transposed [128hc, 256]
            x1T = S2.tile([128, 256], F32, tag=f"x1T{sk}", bufs=1)
            for nt in range(2):
                tr(x1T[:, 128 * nt:128 * (nt + 1)], x123[sk][nt][:, 0:128])
            sT = S2.tile([128, 16, 256], F32, tag=f"sT{sk}", bufs=1)
            cmax = S2.tile([128, 16], F32, tag=f"cmax{sk}", bufs=1)
            for t in range(16):
                x2f = WL.tile([128, 128], F32, tag="x2f")
                nc.sync.dma_start(
                    out=x2f, in_=ag1_out[128 * t:128 * (t + 1),
                                         512 * (sk - 1) + 128:512 * (sk - 1) + 256])
                x2fT = W.tile([128, 128], F32, tag="x2fT")
                tr(x2fT, x2f)
                pss = PS.tile([128, 256], F32, tag="ps")
                pe_obs(x2fT, x1T)
                nc.tensor.matmul(pss, x2fT, x1T, start=True, stop=True)
                dve_obs(pss)
                nc.vector.tensor_reduce(cmax[:, t:t + 1], pss, axis=AX.X, op=ALU.max)
                nc.vector.tensor_copy(sT[:, t, :], pss)
            return sT, cmax

        sT1, cmax1 = skip_softmax_out(1)
        sT2, cmax2 = skip_softmax_out(2)
        cm_in = DR.tile([128, 32], F32)
        nc.sync.dma_start(out=cm_in[:, 0:16], in_=cmax1)
        nc.sync.dma_start(out=cm_in[:, 16:32], in_=cmax2)
        cm_out = DR.tile([128, 32], F32)
        nc.gpsimd.collective_compute("AllReduce", ALU.max, replica_groups=RG,
                                     ins=[cm_in.opt()], outs=[cm_out.opt()])
        ncm = S2.tile([128, 32], F32, tag="ncm", bufs=1)
        nc.sync.dma_start(out=ncm, in_=cm_out[:])
        dve_obs(ncm)
        nc.vector.tensor_scalar_mul(ncm, ncm, -1.0)
        csum = S2.tile([128, 32], F32, tag="csum", bufs=1)
        for sk, sT in ((1, sT1), (2, sT2)):
            for t in range(16):
                act_obs(sT if t == 0 else ncm)
                o = 16 * (sk - 1) + t
                nc.scalar.activation(sT[:, t, :], sT[:, t, :], AF.Exp,
                                     bias=ncm[:, o:o + 1],
                                     accum_out=csum[:, o:o + 1])
        cs_in = DR.tile([128, 32], F32)
        nc.sync.dma_start(out=cs_in[:], in_=csum)
        cs_out = DR.tile([128, 32], F32)
        nc.gpsimd.collective_compute("AllReduce", ALU.add, replica_groups=RG,
                                     ins=[cs_in.opt()], outs=[cs_out.opt()])
        rcs = S2.tile([128, 32], F32, tag="rcs", bufs=1)
        nc.sync.dma_start(out=rcs, in_=cs_out[:])
        dve_obs(rcs)
        nc.vector.reciprocal(rcs, rcs)
        for sk, sT in ((1, sT1), (2, sT2)):
            for t in range(16):
                nc.vector.tensor_scalar_mul(
                    sT[:, t, :], sT[:, t, :],
                    rcs[:, 16 * (sk - 1) + t:16 * (sk - 1) + t + 1])

        # out = featT.T @ X3f + X ; Xs tiles [128,256] x2 ; Ks same
        XsKs = {}
        agK_in = DR.tile([256, 256], F32)
        for sk, sT in ((1, sT1), (2, sT2)):
            outs = []
            for nt in range(2):
                pso = PSB.tile([128, 256], F32, tag="psb")
                for t in range(16):
                    x3f = WL.tile([128, 256], F32, tag="x3f")
                    nc.sync.dma_start(
                        out=x3f, in_=ag1_out[128 * t:128 * (t + 1),
                                             512 * (sk - 1) + 256:512 * sk])
                    pe_obs(x3f, sT if t == 0 else x3f)
                    nc.tensor.matmul(pso, sT[:, t, 128 * nt:128 * (nt + 1)], x3f,
                                     start=(t == 0), stop=(t == 15))
                res = (P if sk == 1 else S2).tile([128, 256], F32, name=f"XsKs{sk}_{nt}", tag=f"XsKs{sk}_{nt}", bufs=1)
                src = nt if sk == 1 else nt + 2
                dve_obs(pso, dt[src])
                nc.vector.tensor_tensor(out=res, in0=pso, in1=dt[src], op=ALU.add)
                outs.append(res)
                if sk == 2:
                    nc.sync.dma_start(out=agK_in[128 * nt:128 * (nt + 1), :], in_=res)
            XsKs[sk] = outs
        agK_out = DR.tile([2048, 256], F32)
        nc.gpsimd.collective_compute("AllGather", ALU.bypass, replica_groups=RG,
                                     ins=[agK_in.opt()], outs=[agK_out.opt()])

        _s2.close()
        # ========================= euclid -> aff_loc =========================
        _s34 = ExitStack()
        S34 = _s34.enter_context(tc.tile_pool(name="s34", bufs=1))
        _s3 = ExitStack()
        S3 = _s3.enter_context(tc.tile_pool(name="s3", bufs=2))
        XsT = S3.tile([128, 2, 2, 128], F32, tag="XsT", bufs=1)  # [d-ch][nt]
        for nt in range(2):
            for ch in range(2):
                tr(XsT[:, ch, nt, :], XsKs[1][nt][:, 128 * ch:128 * (ch + 1)],
                   scale=-2.0)
        xx = S3.tile([128, 2], F32, tag="xx", bufs=1)
        for nt in range(2):
            sq = S3.tile([128, 256], F32, tag="sq")
            dve_obs(XsKs[1][nt])
            nc.vector.tensor_tensor(out=sq, in0=XsKs[1][nt], in1=XsKs[1][nt],
                                    op=ALU.mult)
            nc.vector.tensor_reduce(xx[:, nt:nt + 1], sq, axis=AX.X, op=ALU.add)
        KsT = []
        yyT = S3.tile([128, 16], F32, tag="yyT", bufs=1)
        for ch in range(2):
            KsT.append(S3.tile([128, 2048], F32, name=f"KsT{ch}", tag=f"KsT{ch}", bufs=1))
        for t in range(16):
            kt = WL.tile([128, 256], F32, tag="kt")
            nc.sync.dma_start(out=kt, in_=agK_out[128 * t:128 * (t + 1), :])
            for ch in range(2):
                tr(KsT[ch][:, 128 * t:128 * (t + 1)], kt[:, 128 * ch:128 * (ch + 1)])
            sqk = S3.tile([128, 256], F32, tag="sqk")
            dve_obs(kt)
            nc.vector.tensor_tensor(out=sqk, in0=kt, in1=kt, op=ALU.mult)
            nc.vector.tensor_reduce(yyT[:, t:t + 1], sqk, axis=AX.X, op=ALU.add)
        yy16 = S3.tile([16, 128], F32, tag="yy16", bufs=1)
        tr(yy16, yyT)
        yyr = S3.tile([1, 2048], F32, tag="yyr", bufs=1)
        nc.sync.dma_start(out=yyr.rearrange("o (t p) -> o t p", t=16), in_=yy16)
        aff = []
        for nt in range(2):
            afft = S34.tile([128, 2048], F32, name=f"aff{nt}", tag=f"aff{nt}", bufs=1)
            for fc in range(4):
                psa = PS.tile([128, 512], F32, tag="ps")
                pe_obs(XsT if (nt == 0 and fc == 0) else KsT[0], yyr if fc == 0 else KsT[1])
                nc.tensor.matmul(psa, XsT[:, 0, nt, :],
                                 KsT[0][:, 512 * fc:512 * (fc + 1)],
                                 start=True, stop=False)
                nc.tensor.matmul(psa, XsT[:, 1, nt, :],
                                 KsT[1][:, 512 * fc:512 * (fc + 1)],
                                 start=False, stop=False)
                nc.tensor.matmul(psa, ones1, yyr[:, 512 * fc:512 * (fc + 1)],
                                 start=False, stop=True)
                tmp = S3.tile([128, 512], F32, tag="afftmp")
                dve_obs(psa, xx)
                nc.vector.tensor_scalar(tmp, psa, xx[:, nt:nt + 1], 0.0,
                                        op0=ALU.add, op1=ALU.max)
                act_obs(tmp)
                nc.scalar.activation(afft[:, 512 * fc:512 * (fc + 1)], tmp, AF.Sqrt)
            aff.append(afft)

        _s3.close()
        # ============================== sk3 ==================================
        _s4 = ExitStack()
        S4 = _s4.enter_context(tc.tile_pool(name="s4", bufs=2))
        affT = S4.tile([128, 16, 256], F32, tag="affT", bufs=1)
        for nt in range(2):
            for t in range(16):
                tr(affT[:, t, 128 * nt:128 * (nt + 1)],
                   aff[nt][:, 128 * t:128 * (t + 1)])
        # X12 [128,256] x2 : leaky(aff @ [w1T|w2T] + b) * BN
        x12_3 = []
        ag3_in = DR.tile([256, 2304], F32)
        for nt in range(2):
            ps12 = PSB.tile([128, 256], F32, tag="psb")
            for ch in range(16):
                rch = WL.tile([128, 256], F32, tag="rch")
                nc.sync.dma_start(out=rch, in_=I['sk3_rhs12'][128 * ch:128 * (ch + 1), :])
                pe_obs(rch, affT if ch == 0 else rch)
                nc.tensor.matmul(ps12, affT[:, ch, 128 * nt:128 * (nt + 1)], rch,
                                 start=(ch == 0), stop=False)
            rb = S4.tile([1, 256], F32, tag="rb12")
            nc.sync.dma_start(out=rb, in_=I['sk3_rhs12'][2048:2049, :])
            pe_obs(rb)
            nc.tensor.matmul(ps12, ones1, rb, start=False, stop=True)
            xt = S4.tile([128, 256], F32, name=f"x12_3_{nt}", tag=f"x12_3_{nt}", bufs=1)
            act_obs(ps12)
            nc.scalar.activation(xt, ps12, AF.Lrelu, scale=BN_SCALE, alpha=0.01)
            x12_3.append(xt)
            nc.sync.dma_start(out=ag3_in[128 * nt:128 * (nt + 1), 0:256], in_=xt)
        # X3 local [128, 2048] x2 streamed from sk3_w3T
        b3row = S4.tile([1, 2048], F32, tag="b3row", bufs=1)
        nc.sync.dma_start(out=b3row, in_=I['sk3_w3T'][2048:2049, :])
        x3loc = []
        for nt in range(2):
            x3l = S4.tile([128, 2048], F32, name=f"x3loc{nt}", tag=f"x3loc{nt}", bufs=1)
            x3loc.append(x3l)
        for fc in range(4):
            psw = [PSB.tile([128, 512], F32, name=f"psw{nt}", tag="psb") for nt in range(2)]
            for ch in range(16):
                wch = WL.tile([128, 512], F32, tag="wch")
                nc.sync.dma_start(
                    out=wch, in_=I['sk3_w3T'][128 * ch:128 * (ch + 1),
                                              512 * fc:512 * (fc + 1)])
                pe_obs(wch)
                for nt in range(2):
                    nc.tensor.matmul(psw[nt], affT[:, ch, 128 * nt:128 * (nt + 1)],
                                     wch, start=(ch == 0), stop=False)
            pe_obs(b3row)
            for nt in range(2):
                nc.tensor.matmul(psw[nt], ones1, b3row[:, 512 * fc:512 * (fc + 1)],
                                 start=False, stop=True)
                act_obs(psw[nt])
                nc.scalar.activation(x3loc[nt][:, 512 * fc:512 * (fc + 1)],
                                     psw[nt], AF.Lrelu, scale=BN_SCALE, alpha=0.01)
        for nt in range(2):
            nc.sync.dma_start(out=ag3_in[128 * nt:128 * (nt + 1), 256:2304],
                              in_=x3loc[nt])
        ag3_out = DR.tile([2048, 2304], F32)
        nc.gpsimd.collective_compute("AllGather", ALU.bypass, replica_groups=RG,
                                     ins=[ag3_in.opt()], outs=[ag3_out.opt()])

        # scoresT3, column softmax via AR
        x1T3 = S4.tile([128, 256], F32, tag="x1T3", bufs=1)
        for nt in range(2):
            tr(x1T3[:, 128 * nt:128 * (nt + 1)], x12_3[nt][:, 0:128])
        sT3 = S4.tile([128, 16, 256], F32, tag="sT3", bufs=1)
        cmax3 = S4.tile([128, 16], F32, tag="cmax3", bufs=1)
        for t in range(16):
            x2f = WL.tile([128, 128], F32, tag="x2f3")
            nc.sync.dma_start(out=x2f,
                              in_=ag3_out[128 * t:128 * (t + 1), 128:256])
            x2fT = W.tile([128, 128], F32, tag="x2f3T")
            tr(x2fT, x2f)
            pss = PS.tile([128, 256], F32, tag="ps")
            pe_obs(x2fT, x1T3)
            nc.tensor.matmul(pss, x2fT, x1T3, start=True, stop=True)
            dve_obs(pss)
            nc.vector.tensor_reduce(cmax3[:, t:t + 1], pss, axis=AX.X, op=ALU.max)
            nc.vector.tensor_copy(sT3[:, t, :], pss)
        cm3_in = DR.tile([128, 16], F32)
        nc.sync.dma_start(out=cm3_in[:], in_=cmax3)
        cm3_out = DR.tile([128, 16], F32)
        nc.gpsimd.collective_compute("AllReduce", ALU.max, replica_groups=RG,
                                     ins=[cm3_in.opt()], outs=[cm3_out.opt()])
        ncm3 = S4.tile([128, 16], F32, tag="ncm3", bufs=1)
        nc.sync.dma_start(out=ncm3, in_=cm3_out[:])
        dve_obs(ncm3)
        nc.vector.tensor_scalar_mul(ncm3, ncm3, -1.0)
        csum3 = S4.tile([128, 16], F32, tag="csum3", bufs=1)
        for t in range(16):
            act_obs(sT3 if t == 0 else ncm3)
            nc.scalar.activation(sT3[:, t, :], sT3[:, t, :], AF.Exp,
                                 bias=ncm3[:, t:t + 1],
                                 accum_out=csum3[:, t:t + 1])
        cs3_in = DR.tile([128, 16], F32)
        nc.sync.dma_start(out=cs3_in[:], in_=csum3)
        cs3_out = DR.tile([128, 16], F32)
        nc.gpsimd.collective_compute("AllReduce", ALU.add, replica_groups=RG,
                                     ins=[cs3_in.opt()], outs=[cs3_out.opt()])
        rcs3 = S4.tile([128, 16], F32, tag="rcs3", bufs=1)
        nc.sync.dma_start(out=rcs3, in_=cs3_out[:])
        dve_obs(rcs3)
        nc.vector.reciprocal(rcs3, rcs3)
        for t in range(16):
            nc.vector.tensor_scalar_mul(sT3[:, t, :], sT3[:, t, :],
                                        rcs3[:, t:t + 1])
        # aff_s = featT.T @ X3f + aff
        affs = []
        for nt in range(2):
            affs.append(P.tile([128, 2048], F32, name=f"affs{nt}", tag=f"affs{nt}", bufs=1))
        for fc in range(4):
            pso = [PSB.tile([128, 512], F32, name=f"pso{nt}", tag="psb") for nt in range(2)]
            for t in range(16):
                x3f = WL.tile([128, 512], F32, tag="x3f3")
                nc.sync.dma_start(
                    out=x3f, in_=ag3_out[128 * t:128 * (t + 1),
                                         256 + 512 * fc:256 + 512 * (fc + 1)])
                pe_obs(x3f, sT3 if (fc == 0 and t == 0) else x3f)
                for nt in range(2):
                    nc.tensor.matmul(pso[nt], sT3[:, t, 128 * nt:128 * (nt + 1)],
                                     x3f, start=(t == 0), stop=(t == 15))
            for nt in range(2):
                dve_obs(pso[nt], aff[nt])
                nc.vector.tensor_tensor(
                    out=affs[nt][:, 512 * fc:512 * (fc + 1)], in0=pso[nt],
                    in1=aff[nt][:, 512 * fc:512 * (fc + 1)], op=ALU.add)

        _s4.close()
        _s34.close()
        _s5 = ExitStack()
        S5 = _s5.enter_context(tc.tile_pool(name="s5", bufs=2))
        affsT = S5.tile([128, 16, 256], F32, tag="affsT", bufs=1)
        for nt in range(2):
            for t in range(16):
                tr(affsT[:, t, 128 * nt:128 * (nt + 1)],
                   affs[nt][:, 128 * t:128 * (t + 1)])

        # ========================== mask_scores ==============================
        x1m = S5.tile([128, 2, 128], F32, tag="x1m", bufs=1)
        x2m = S5.tile([128, 2, 128], F32, tag="x2m", bufs=1)
        for nt in range(2):
            psm = PSB.tile([128, 256], F32, tag="psb")
            for ch in range(16):
                rch = WL.tile([128, 256], F32, tag="rchm")
                nc.sync.dma_start(out=rch,
                                  in_=I['mk_rhs12'][128 * ch:128 * (ch + 1), :])
                pe_obs(rch, affsT if ch == 0 else rch)
                nc.tensor.matmul(psm, affsT[:, ch, 128 * nt:128 * (nt + 1)], rch,
                                 start=(ch == 0), stop=False)
            rbm = S5.tile([1, 256], F32, tag="rbm", bufs=1)
            nc.sync.dma_start(out=rbm, in_=I['mk_rhs12'][2048:2049, :])
            pe_obs(rbm)
            nc.tensor.matmul(psm, ones1, rbm, start=False, stop=True)
            act_obs(psm)
            nc.scalar.activation(x1m[:, nt, :], psm[:, 0:128], AF.Lrelu, alpha=0.01)
            nc.scalar.activation(x2m[:, nt, :], psm[:, 128:256], AF.Tanh)
        psM = PS.tile([128, 128], F32, tag="ps")
        pe_obs(x1m, x2m)
        for nt in range(2):
            nc.tensor.matmul(psM, x1m[:, nt, :], x2m[:, nt, :],
                             start=(nt == 0), stop=(nt == 1))
        Msb = S5.tile([128, 128], F32, tag="Msb", bufs=1)
        dve_obs(psM)
        nc.vector.tensor_copy(Msb, psM)
        arM_in = DR.tile([128, 128], F32)
        nc.sync.dma_start(out=arM_in[:], in_=Msb)
        arM_out = DR.tile([128, 128], F32)
        nc.gpsimd.collective_compute("AllReduce", ALU.add, replica_groups=RG,
                                     ins=[arM_in.opt()], outs=[arM_out.opt()])
        Mfull = S5.tile([128, 128], F32, tag="Mfull", bufs=1)
        nc.sync.dma_start(out=Mfull, in_=arM_out[:])
        MT = S5.tile([128, 128], F32, tag="MT", bufs=1)
        tr(MT, Mfull)
        mk3 = S5.tile([128, 2048], F32, tag="mk3", bufs=1)
        nc.sync.dma_start(out=mk3, in_=I['mk_w3T'][0:128, :])
        mk3b = S5.tile([1, 2048], F32, tag="mk3b", bufs=1)
        nc.sync.dma_start(out=mk3b, in_=I['mk_w3T'][128:129, :])
        w4 = S5.tile([128, 1], F32, tag="w4", bufs=1)
        nc.sync.dma_start(out=w4, in_=I['w4col'][:, :])
        A3 = S5.tile([128, 2048], F32, tag="A3", bufs=1)
        srow = S5.tile([1, 2048], F32, tag="srow", bufs=1)
        for fc in range(4):
            psA = PS.tile([128, 512], F32, tag="ps")
            pe_obs(MT if fc == 0 else mk3, mk3 if fc == 0 else mk3b)
            nc.tensor.matmul(psA, MT, mk3[:, 512 * fc:512 * (fc + 1)],
                             start=True, stop=False)
            nc.tensor.matmul(psA, ones1,
                             mk3b[:, 512 * fc:512 * (fc + 1)],
                             start=False, stop=True)
            act_obs(psA)
            nc.scalar.activation(A3[:, 512 * fc:512 * (fc + 1)], psA, AF.Lrelu,
                                 alpha=0.01)
            psS = PS.tile([1, 512], F32, tag="ps")
            pe_obs(A3 if fc == 0 else w4, w4 if fc == 0 else A3)
            nc.tensor.matmul(psS, w4, A3[:, 512 * fc:512 * (fc + 1)],
                             start=True, stop=True)
            dve_obs(psS)
            nc.vector.tensor_scalar_add(srow[:, 512 * fc:512 * (fc + 1)], psS,
                                        float(scal['b4']))
        smax = S5.tile([1, 1], F32, tag="smax", bufs=1)
        nc.vector.tensor_reduce(smax, srow, axis=AX.X, op=ALU.max)
        nc.vector.tensor_scalar_mul(smax, smax, -1.0)
        esr = S5.tile([1, 2048], F32, tag="esr", bufs=1)
        ssum = S5.tile([1, 1], F32, tag="ssum", bufs=1)
        act_obs(srow)
        nc.scalar.activation(esr, srow, AF.Exp, bias=smax[:, :], accum_out=ssum)
        rss = S5.tile([1, 1], F32, tag="rss", bufs=1)
        dve_obs(esr)
        nc.vector.reciprocal(rss, ssum)
        scores = P.tile([1, 2048], F32, tag="scores")
        nc.vector.tensor_scalar_mul(scores, esr, rss[:, :])

        _s5.close()
        # ============================ ranks ==================================
        _s6 = ExitStack()
        S6 = _s6.enter_context(tc.tile_pool(name="s6", bufs=2))
        sc_d = DR.tile([1, 2048], F32)
        nc.sync.dma_start(out=sc_d[:], in_=scores)
        Sbc = S6.tile([128, 2048], F32, tag="Sbc", bufs=1)
        nc.sync.dma_start(out=Sbc, in_=sc_d.opt()[0, :].partition_broadcast(128))
        iotabc = S6.tile([128, 2048], F32, tag="iotabc", bufs=1)
        nc.sync.dma_start(out=iotabc, in_=I['iota'][0, :].partition_broadcast(128))
        jl = S6.tile([128, 2], F32, tag="jl", bufs=1)
        nc.sync.dma_start(out=jl, in_=I['jidx'][:, 0].rearrange("(t p) -> p t", t=2))
        sTl = S6.tile([128, 2], F32, tag="sTl", bufs=1)
        # local score columns: transpose scores slices at this core's offset
        # offset depends on core: use jidx-based gather instead -> simpler:
        # sTl[p, t] = scores[0, jidx[p + 128 t]] ; jidx rows are contiguous
        # so it's a plain slice of scores at [cNL + 128t : cNL + 128(t+1)].
        # We cannot know c at build time; use partition_id? Instead compute
        # via compare trick: sTl = reduce over free of Sbc * onehot(iota==j).
        for tt in range(2):
            eqj = S6.tile([128, 2048], F32, tag="eqj", bufs=1)
            dve_obs(iotabc if tt == 0 else jl, jl if tt == 0 else iotabc)
            nc.vector.tensor_scalar(eqj, iotabc, jl[:, tt:tt + 1], None,
                                    op0=ALU.is_equal)
            nc.vector.tensor_tensor(out=eqj, in0=eqj, in1=Sbc, op=ALU.mult)
            nc.vector.tensor_reduce(sTl[:, tt:tt + 1], eqj, axis=AX.X, op=ALU.add)
        rankp = S6.tile([128, 2], F32, tag="rankp", bufs=1)
        for tt in range(2):
            G = S6.tile([128, 2048], F32, tag="G", bufs=1)
            nc.vector.tensor_scalar(G, Sbc, sTl[:, tt:tt + 1], None, op0=ALU.is_gt)
            E1 = S6.tile([128, 2048], F32, tag="E1", bufs=1)
            nc.vector.tensor_scalar(E1, Sbc, sTl[:, tt:tt + 1], None,
                                    op0=ALU.is_equal)
            L1 = S6.tile([128, 2048], F32, tag="L1", bufs=1)
            nc.vector.tensor_scalar(L1, iotabc, jl[:, tt:tt + 1], None,
                                    op0=ALU.is_lt)
            nc.vector.tensor_tensor(out=E1, in0=E1, in1=L1, op=ALU.mult)
            nc.vector.tensor_tensor(out=G, in0=G, in1=E1, op=ALU.add)
            nc.vector.tensor_reduce(rankp[:, tt:tt + 1], G, axis=AX.X, op=ALU.add)
        agR_in = DR.tile([256, 1], F32)
        nc.sync.dma_start(out=agR_in.opt()[:, 0].rearrange("(t p) -> p t", t=2),
                          in_=rankp)
        agR_out = DR.tile([2048, 1], F32)
        nc.gpsimd.collective_compute("AllGather", ALU.bypass, replica_groups=RG,
                                     ins=[agR_in.opt()], outs=[agR_out.opt()])
        rkbc = S6.tile([128, 2048], F32, tag="rkbc", bufs=1)
        nc.sync.dma_start(out=rkbc, in_=agR_out.opt()[:, 0].partition_broadcast(128))
        # mask bias row [128, 2048] : (rank >= S) * -1e9
        mb = P.tile([128, 2048], F32, tag="mb")
        dve_obs(rkbc)
        nc.vector.tensor_scalar(mb, rkbc, float(S), -1e9, op0=ALU.is_ge,
                                op1=ALU.mult)
        # one-hot PT and W'T
        kA = S6.tile([128, 1], F32, tag="kA", bufs=1)
        nc.sync.dma_start(out=kA, in_=I['kidx'][0:128, :])
        kB = S6.tile([76, 1], F32, tag="kB", bufs=1)
        nc.sync.dma_start(out=kB, in_=I['kidx'][128:204, :])
        PTa = S6.tile([128, 2048], F32, tag="PTa", bufs=1)
        dve_obs(kA, kB)
        nc.vector.tensor_scalar(PTa, rkbc, kA[:, :], None, op0=ALU.is_equal)
        PTb = S6.tile([76, 2048], F32, tag="PTb", bufs=1)
        nc.vector.tensor_scalar(PTb, rkbc[0:76, :], kB[:, :], None,
                                op0=ALU.is_equal)
        fwA = S6.tile([128, 256], F32, tag="fwA", bufs=1)
        nc.sync.dma_start(out=fwA, in_=I['fw2T'][0:128, :])
        fwB = S6.tile([76, 256], F32, tag="fwB", bufs=1)
        nc.sync.dma_start(out=fwB, in_=I['fw2T'][128:204, :])
        WpT = P.tile([128, 16, 256], F32, tag="WpT")
        pe_obs(PTa, PTb, fwA, fwB)
        for jt in range(16):
            psW = PS.tile([128, 256], F32, tag="ps")
            nc.tensor.matmul(psW, PTa[:, 128 * jt:128 * (jt + 1)], fwA,
                             start=True, stop=False)
            nc.tensor.matmul(psW, PTb[:, 128 * jt:128 * (jt + 1)], fwB,
                             start=False, stop=True)
            dve_obs(psW if jt == 0 else psW)
            nc.vector.tensor_copy(WpT[:, jt, :], psW)

        _s6.close()
        # ===================== fuse + attention ==============================
        _s7 = ExitStack()
        S7 = _s7.enter_context(tc.tile_pool(name="s7", bufs=2))
        fw1A = S7.tile([128, 256], F32, tag="fw1A", bufs=1)
        nc.sync.dma_start(out=fw1A, in_=I['fuse_w1T'][0:128, :])
        fw1B = S7.tile([128, 256], F32, tag="fw1B", bufs=1)
        nc.sync.dma_start(out=fw1B, in_=I['fuse_w1T'][128:256, :])
        fw1C = S7.tile([1, 256], F32, tag="fw1C", bufs=1)
        nc.sync.dma_start(out=fw1C, in_=I['fuse_w1T'][256:257, :])
        EhT = S7.tile([128, 16, 256], F32, tag="EhT", bufs=1)
        for nt in range(2):
            madd = S7.tile([128, 2048], F32, tag="madd", bufs=1)
            dve_obs(affs[nt], mb)
            nc.vector.tensor_tensor(out=madd, in0=affs[nt], in1=mb, op=ALU.add)
            rmax = S7.tile([128, 1], F32, tag="rmax", bufs=1)
            nc.vector.tensor_reduce(rmax, madd, axis=AX.X, op=ALU.max)
            nc.vector.tensor_scalar_mul(rmax, rmax, -1.0)
            Eh = S7.tile([128, 2048], F32, tag="Eh", bufs=1)
            rsum = S7.tile([128, 1], F32, tag="rsum", bufs=1)
            act_obs(madd)
            nc.scalar.activation(Eh, madd, AF.Exp, bias=rmax[:, :], accum_out=rsum)
            rrs = S7.tile([128, 1], F32, tag="rrs", bufs=1)
            dve_obs(Eh)
            nc.vector.reciprocal(rrs, rsum)
            nc.vector.tensor_scalar_mul(Eh, Eh, rrs[:, :])
            for t in range(16):
                tr(EhT[:, t, 128 * nt:128 * (nt + 1)],
                   Eh[:, 128 * t:128 * (t + 1)])
        Bf = S7.tile([128, 2, 257], F32, tag="Bf", bufs=1)
        for nt in range(2):
            psF = PSB.tile([128, 256], F32, tag="psb")
            pe_obs(EhT if nt == 0 else WpT, WpT if nt == 0 else EhT,
                   fw1A if nt == 0 else fw1B, fw1B if nt == 0 else fw1C,
                   fw1C if nt == 0 else fw1A)
            for t in range(16):
                nc.tensor.matmul(psF, EhT[:, t, 128 * nt:128 * (nt + 1)],
                                 WpT[:, t, :], start=(t == 0), stop=False)
            src = nt
            nc.tensor.matmul(psF, dtT[src][:, 0, :], fw1A, start=False, stop=False)
            nc.tensor.matmul(psF, dtT[src][:, 1, :], fw1B, start=False, stop=False)
            nc.tensor.matmul(psF, ones1, fw1C, start=False, stop=True)
            act_obs(psF)
            nc.scalar.activation(Bf[:, nt, 0:256], psF, AF.Lrelu, alpha=0.01)
            nc.vector.memset(Bf[:, nt, 256:257], 1.0)


        a1vuT = []
        for nm, sl in (('a1vuT', None),):
            a = S7.tile([128, 256], F32, tag="a1vuTA", bufs=1)
            nc.sync.dma_start(out=a, in_=I['a1vuT'][0:128, :])
            b = S7.tile([128, 256], F32, tag="a1vuTB", bufs=1)
            nc.sync.dma_start(out=b, in_=I['a1vuT'][128:256, :])
            c = S7.tile([1, 256], F32, tag="a1vuTC", bufs=1)
            nc.sync.dma_start(out=c, in_=I['a1vuT'][256:257, :])
            a1vuT = [a, b, c]
        a1fe = []
        for part, shp, tg in (((0, 128), (128, 256), "a1feA"),
                              ((128, 256), (128, 256), "a1feB"),
                              ((256, 257), (1, 256), "a1feC")):
            t = S7.tile(list(shp), F32, name=tg, tag=tg, bufs=1)
            nc.sync.dma_start(out=t, in_=I['a1feT'][part[0]:part[1], :])
            a1fe.append(t)
        aw1 = S7.tile([128, 1], F32, tag="aw1", bufs=1)
        nc.sync.dma_start(out=aw1, in_=I['aw1col'][:, :])

        BfT = S7.tile([128, 2, 2, 128], F32, tag="BfT", bufs=1)
        for nt in range(2):
            for ch in range(2):
                tr(BfT[:, ch, nt, :], Bf[:, nt, 128 * ch:128 * (ch + 1)])

        def a1_chunks(nt):
            return [(BfT[:, 0, nt, :], a1fe[0]), (BfT[:, 1, nt, :], a1fe[1])]

        # a1 attention; bias row handled via the ones matmul against a1fe[2]
        def attn_run(chunks_fn, fe_bias_row, vuT, awcol, awb, tagn):
            awb_t = S7.tile([128, 1], F32, name=f"awb{tagn}", tag=f"awb{tagn}", bufs=1)
            nc.vector.memset(awb_t, float(awb))
            Ht = S7.tile([128, 2, 257], F32, name=f"H{tagn}", tag=f"H{tagn}", bufs=1)
            for nt in range(2):
                psH = PSB.tile([128, 256], F32, tag="psb")
                first = True
                for lhsT, rhs in chunks_fn(nt):
                    pe_obs(lhsT, rhs)
                    nc.tensor.matmul(psH, lhsT, rhs, start=first, stop=False)
                    first = False
                pe_obs(fe_bias_row)
                nc.tensor.matmul(psH, ones1, fe_bias_row, start=False, stop=True)
                act_obs(psH)
                nc.scalar.activation(Ht[:, nt, 0:256], psH, AF.Lrelu, alpha=0.01)
                nc.vector.memset(Ht[:, nt, 256:257], 1.0)
            HT = S7.tile([128, 2, 2, 128], F32, name=f"HT{tagn}", tag=f"HT{tagn}", bufs=1)
            for nt in range(2):
                for ch in range(2):
                    tr(HT[:, ch, nt, :], Ht[:, nt, 128 * ch:128 * (ch + 1)])
            ea = S7.tile([128, 2], F32, name=f"ea{tagn}", tag=f"ea{tagn}", bufs=1)
            for nt in range(2):
                psV = PS.tile([128, 256], F32, tag="ps")
                pe_obs(HT, vuT[0], vuT[1], vuT[2])
                nc.tensor.matmul(psV, HT[:, 0, nt, :], vuT[0], start=True,
                                 stop=False)
                nc.tensor.matmul(psV, HT[:, 1, nt, :], vuT[1], start=False,
                                 stop=False)
                nc.tensor.matmul(psV, ones1, vuT[2], start=False, stop=True)
                AV = S7.tile([128, 128], F32, tag="AV")
                AU = S7.tile([128, 128], F32, tag="AU")
                act_obs(psV)
                nc.scalar.activation(AV, psV[:, 0:128], AF.Tanh)
                nc.scalar.activation(AU, psV[:, 128:256], AF.Lrelu, alpha=0.01)
                PP = S7.tile([128, 128], F32, tag="PP")
                dve_obs(AV, AU)
                nc.vector.tensor_tensor(out=PP, in0=AV, in1=AU, op=ALU.mult)
                PPT = S7.tile([128, 128], F32, tag="PPT")
                tr(PPT, PP)
                psa2 = PS.tile([128, 1], F32, tag="ps")
                pe_obs(PPT, awcol)
                nc.tensor.matmul(psa2, PPT, awcol, start=True, stop=True)
                aout = S7.tile([128, 1], F32, tag="aout")
                act_obs(psa2)
                nc.scalar.activation(aout, psa2, AF.Lrelu, bias=awb_t[:, :],
                                     alpha=0.01)
                nc.scalar.activation(ea[:, nt:nt + 1], aout, AF.Exp)
            psU = PS.tile([1, 257], F32, tag="ps")
            pe_obs(ea, Ht)
            for nt in range(2):
                nc.tensor.matmul(psU, ea[:, nt:nt + 1], Ht[:, nt, :],
                                 start=(nt == 0), stop=(nt == 1))
            usb = S7.tile([1, 257], F32, name=f"usb{tagn}", tag=f"usb{tagn}", bufs=1)
            dve_obs(psU)
            nc.vector.tensor_copy(usb, psU)
            return ea, usb

        ea1, us1 = attn_run(a1_chunks, a1fe[2], a1vuT, aw1, scal['awb1'], "a1")
        ar1_in = DR.tile([1, 257], F32)
        nc.sync.dma_start(out=ar1_in[:], in_=us1)
        ar1_out = DR.tile([1, 257], F32)
        nc.gpsimd.collective_compute("AllReduce", ALU.add, replica_groups=RG,
                                     ins=[ar1_in.opt()], outs=[ar1_out.opt()])
        u1 = S7.tile([1, 257], F32, tag="u1", bufs=1)
        nc.sync.dma_start(out=u1, in_=ar1_out[:])
        r1 = S7.tile([1, 1], F32, tag="r1", bufs=1)
        dve_obs(u1)
        nc.vector.reciprocal(r1, u1[:, 256:257])
        un1 = S7.tile([1, 256], F32, tag="un1", bufs=1)
        nc.vector.tensor_scalar_mul(un1, u1[:, 0:256], r1[:, :])
        uT1 = S7.tile([128, 2], F32, tag="uT1", bufs=1)
        for ch in range(2):
            tr(uT1[:, ch:ch + 1], un1[:, 128 * ch:128 * (ch + 1)])
        a1em = []
        for part, shp, tg in (((0, 128), (128, 256), "a1emA"),
                              ((128, 256), (128, 256), "a1emB"),
                              ((256, 257), (1, 256), "a1emC")):
            t = S7.tile(list(shp), F32, name=tg, tag=tg, bufs=1)
            nc.sync.dma_start(out=t, in_=I['a1embT'][part[0]:part[1], :])
            a1em.append(t)
        psM1 = PS.tile([1, 256], F32, tag="ps")
        pe_obs(uT1, a1em[0], a1em[1], a1em[2])
        nc.tensor.matmul(psM1, uT1[:, 0:1], a1em[0], start=True, stop=False)
        nc.tensor.matmul(psM1, uT1[:, 1:2], a1em[1], start=False, stop=False)
        nc.tensor.matmul(psM1, ones1[0:1, 0:1], a1em[2], start=False, stop=True)
        M1 = S7.tile([1, 256], F32, tag="M1", bufs=1)
        act_obs(psM1)
        nc.scalar.activation(M1, psM1, AF.Lrelu, alpha=0.01)
        M1T = S7.tile([128, 2], F32, tag="M1T", bufs=1)
        for ch in range(2):
            tr(M1T[:, ch:ch + 1], M1[:, 128 * ch:128 * (ch + 1)])
        ones128 = S7.tile([128, 128], F32, tag="ones128", bufs=1)
        nc.vector.memset(ones128, 1.0)
        M1bc = S7.tile([128, 2, 128], F32, tag="M1bc", bufs=1)
        dve_obs(M1T)
        for ch in range(2):
            nc.vector.tensor_scalar_mul(M1bc[:, ch, :], ones128,
                                        M1T[:, ch:ch + 1])
        a2fe = []
        for part, shp, tg in (((0, 128), (128, 256), "a2feA"),
                              ((128, 256), (128, 256), "a2feB"),
                              ((256, 384), (128, 256), "a2feC"),
                              ((384, 512), (128, 256), "a2feD"),
                              ((512, 513), (1, 256), "a2feE")):
            t = S7.tile(list(shp), F32, name=tg, tag=tg, bufs=1)
            nc.sync.dma_start(out=t, in_=I['a2feT'][part[0]:part[1], :])
            a2fe.append(t)
        a2vuT = []
        for part, shp, tg in (((0, 128), (128, 256), "a2vuA"),
                              ((128, 256), (128, 256), "a2vuB"),
                              ((256, 257), (1, 256), "a2vuC")):
            t = S7.tile(list(shp), F32, name=tg, tag=tg, bufs=1)
            nc.sync.dma_start(out=t, in_=I['a2vuT'][part[0]:part[1], :])
            a2vuT.append(t)
        aw2 = S7.tile([128, 1], F32, tag="aw2", bufs=1)
        nc.sync.dma_start(out=aw2, in_=I['aw2col'][:, :])

        def a2_chunks(nt):
            return [(BfT[:, 0, nt, :], a2fe[0]), (BfT[:, 1, nt, :], a2fe[1]),
                    (M1bc[:, 0, :], a2fe[2]), (M1bc[:, 1, :], a2fe[3])]

        ea2, us2 = attn_run(a2_chunks, a2fe[4], a2vuT, aw2, scal['awb2'], "a2")
        ar2_in = DR.tile([1, 257], F32)
        nc.sync.dma_start(out=ar2_in[:], in_=us2)
        ar2_out = DR.tile([1, 257], F32)
        nc.gpsimd.collective_compute("AllReduce", ALU.add, replica_groups=RG,
                                     ins=[ar2_in.opt()], outs=[ar2_out.opt()])
        u2 = S7.tile([1, 257], F32, tag="u2", bufs=1)
        nc.sync.dma_start(out=u2, in_=ar2_out[:])
        r2 = S7.tile([1, 1], F32, tag="r2", bufs=1)
        dve_obs(u2)
        nc.vector.reciprocal(r2, u2[:, 256:257])
        # A output: ea2 * (1/s2)
        r2d = DR.tile([1, 1], F32)
        nc.sync.dma_start(out=r2d[:], in_=r2)
        r2b = S7.tile([128, 1], F32, tag="r2b", bufs=1)
        nc.sync.dma_start(out=r2b, in_=r2d.opt()[0, :].partition_broadcast(128))
        An = S7.tile([128, 2], F32, tag="An", bufs=1)
        dve_obs(r2b, ea2)
        for nt in range(2):
            nc.vector.tensor_scalar_mul(An[:, nt:nt + 1], ea2[:, nt:nt + 1],
                                        r2b[:, :])
        nc.sync.dma_start(out=out_A[:, 0].rearrange("(t p) -> p t", t=2), in_=An)
        # bvec
        un2 = S7.tile([1, 256], F32, tag="un2", bufs=1)
        nc.vector.tensor_scalar_mul(un2, u2[:, 0:256], r2[:, :])
        uT2 = S7.tile([128, 2], F32, tag="uT2", bufs=1)
        for ch in range(2):
            tr(uT2[:, ch:ch + 1], un2[:, 128 * ch:128 * (ch + 1)])
        a2em = []
        for part, shp, tg in (((0, 128), (128, 512), "a2emA"),
                              ((128, 256), (128, 512), "a2emB"),
                              ((256, 257), (1, 512), "a2emC")):
            t = S7.tile(list(shp), F32, name=tg, tag=tg, bufs=1)
            nc.sync.dma_start(out=t, in_=I['a2embT'][part[0]:part[1], :])
            a2em.append(t)
        psBv = PS.tile([1, 512], F32, tag="ps")
        pe_obs(uT2, a2em[0], a2em[1], a2em[2])
        nc.tensor.matmul(psBv, uT2[:, 0:1], a2em[0], start=True, stop=False)
        nc.tensor.matmul(psBv, uT2[:, 1:2], a2em[1], start=False, stop=False)
        nc.tensor.matmul(psBv, ones1[0:1, 0:1], a2em[2], start=False, stop=True)
        bvec = S7.tile([1, 512], F32, tag="bvec", bufs=1)
        act_obs(psBv)
        nc.scalar.activation(bvec, psBv, AF.Lrelu, alpha=0.01)
        bvT = S7.tile([128, 4], F32, tag="bvT", bufs=1)
        for ch in range(4):
            tr(bvT[:, ch:ch + 1], bvec[:, 128 * ch:128 * (ch + 1)])
        semb = []
        for part, shp, tg in (((0, 128), (128, 256), "sembA"),
                              ((128, 256), (128, 256), "sembB"),
                              ((256, 384), (128, 256), "sembC"),
                              ((384, 512), (128, 256), "sembD"),
                              ((512, 513), (1, 256), "sembE")):
            t = S7.tile(list(shp), F32, name=tg, tag=tg, bufs=1)
            nc.sync.dma_start(out=t, in_=I['sembT'][part[0]:part[1], :])
            semb.append(t)
        psB2 = PS.tile([1, 256], F32, tag="ps")
        pe_obs(bvT, semb[0], semb[1], semb[2], semb[3], semb[4])
        for ch in range(4):
            nc.tensor.matmul(psB2, bvT[:, ch:ch + 1], semb[ch],
                             start=(ch == 0), stop=False)
        nc.tensor.matmul(psB2, ones1[0:1, 0:1], semb[4], start=False, stop=True)
        bv2 = S7.tile([1, 256], F32, tag="bv2", bufs=1)
        act_obs(psB2)
        nc.scalar.activation(bv2, psB2, AF.Lrelu, alpha=0.01)
        bvT2 = S7.tile([128, 2], F32, tag="bvT2", bufs=1)
        for ch in range(2):
            tr(bvT2[:, ch:ch + 1], bv2[:, 128 * ch:128 * (ch + 1)])
        cls = []
        for part, shp, tg in (((0, 128), (128, 1), "clsA"),
                              ((128, 256), (128, 1), "clsB"),
                              ((256, 257), (1, 1), "clsC")):
            t = S7.tile(list(shp), F32, name=tg, tag=tg, bufs=1)
            nc.sync.dma_start(out=t, in_=I['clsT'][part[0]:part[1], :])
            cls.append(t)
        psY = PS.tile([1, 1], F32, tag="ps")
        pe_obs(bvT2, cls[0], cls[1], cls[2])
        nc.tensor.matmul(psY, bvT2[:, 0:1], cls[0], start=True, stop=False)
        nc.tensor.matmul(psY, bvT2[:, 1:2], cls[1], start=False, stop=False)
        nc.tensor.matmul(psY, ones1[0:1, 0:1], cls[2], start=False, stop=True)
        ysb = S7.tile([1, 1], F32, tag="ysb", bufs=1)
        act_obs(psY)
        nc.scalar.activation(ysb, psY, AF.Sigmoid)
        dve_obs(ysb)
        nc.vector.tensor_scalar(ysb, ysb, 1e-5, 1.0 - 1e-5, op0=ALU.max,
                                op1=ALU.min)
        nc.sync.dma_start(out=out_Y[:, :], in_=ysb)
        _s7.close()
    nc.compile()
    return nc


_CACHE = {}


def kernel(**inputs):
    from concourse.bass_utils import run_bass_kernel_spmd
    h, percore, scal = host_prep(inputs)
    key = tuple(sorted(scal.items()))
    if key not in _CACHE:
        _CACHE[key] = build_nc(scal)
    nc = _CACHE[key]
    res = run_bass_kernel_spmd(nc, percore, list(range(NC))).results
    Y = np.asarray(res[0]['Y']).reshape(1, 1).astype(F32NP)
    A = np.concatenate([np.asarray(res[c]['Apart']).reshape(-1) for c in range(NC)])
    return Y, A[None, :].astype(F32NP)
